# revision 1
# baseline (speedup 1.0000x reference)
"""NSMCell message-passing kernel for 8 Trainium2 NeuronCores.

Contract: kernel(**inputs) takes the FULL unsharded inputs and returns the
FULL (N,) float32 output, matching reference.reference().

Design (see v3 docstring for the core ideas):
  * host pre-gating -> graph-agnostic dense edge/node streams, shared
    stationary weights
  * elu in ONE ScalarE pass via a patched activation table (exp -> elu)
  * w.elu(z) dots as moving-stream reduce matmuls with one-hot [128,32]
    stationaries packing 96 accumulator rows per PSUM bank
Pipeline details (baseline 215.6us -> this version 179.9us on HW):
  * depth-3 software pipelining (reduce lags three tiles) so the PE never
    waits on ScalarE's elu; ScalarE act-table prefetched via a tiny
    warm-up activation at startup
  * DMA emission order tuned for startup (W_edge + first 512 edge cols
    first); 5 edge pieces kept in flight; node-attr sub-DMAs spread
    one-per-edge-piece so they never burst-delay the edge stream
  * accumulator evacuated by the (otherwise idle) VectorE and written out
    as 32-row DMA pieces across queues; the final piece is small so the
    exposed drain tail is ~1us
"""

import json
import os
import shutil
import struct
import sys
import types

import numpy as np

# ---------------------------------------------------------------------------
# problem constants (hardcoded per contract)
N, P, H, E, B = 100000, 4, 128, 1000000, 64
NCORES = 8
TZ = 512            # PSUM bank (f32 cols) = reduce block
ZT = 1536           # z tile cols (3 PSUM banks)
EC = 125952         # per-core padded edge count  (= 82*1536 = 246*512)
NC = 12800          # per-core padded node count  (= 8*1536 + 512 = 25*512)
ETILES = EC // ZT   # 82
NROWS = EC // TZ + NC // TZ  # 246 + 25 = 271 accumulator rows per core
NODE_EVERY = 9      # a node tile is emitted after every 9th edge tile
OH = 32             # one-hot window width (legal matmul out bases: 0/32/64)
RPB = 96            # accumulator rows packed per PSUM bank (3 windows)
PIPE = 3            # reduce lags this many tiles behind the z matmul


# ---------------------------------------------------------------------------
def _install_ntff_hook():
    """Allow BASS_TRACE=1 profiling under axon (test.py); harmless otherwise."""
    try:
        from antenv.axon_hooks import get_axon_ntff_profile_hook  # noqa: F401
        return
    except ImportError:
        pass
    try:
        from trn_agent_boot.trn_boot import _ntff_profile_via_ctypes
        hook = _ntff_profile_via_ctypes("/opt/axon/libaxon_pjrt.so")
    except Exception:
        hook = None
    mod = types.ModuleType("antenv.axon_hooks")
    _state = {"hook": hook}
    mod.get_axon_ntff_profile_hook = lambda: _state["hook"]
    mod.set_axon_ntff_profile_hook = lambda h: _state.__setitem__("hook", h)
    sys.modules["antenv.axon_hooks"] = mod
    try:
        import antenv
        antenv.axon_hooks = mod
    except ImportError:
        pass


# ---------------------------------------------------------------------------
def _build_elu_act_root(dst_dir: str) -> str:
    """Patch activation set `exp_and_others` so `exp` evaluates elu.

    The ACT engine is a piecewise-cubic spline evaluator; bucket entries
    are [d0,d1,d2,d3,x0,pad,pad,pad] f32 with y = d0 + (x-x0)*(d1 +
    (x-x0)*(d2 + (x-x0)*d3)). exp owns buckets 0..780: negative-x0 ->
    d0 -= 1 (e^x -> e^x - 1); positive-x0 -> exact identity [x0,1,0,0,x0];
    specials 777..780 are the small/large-signal buckets. The patched dir
    is passed to the bass->walrus compile via BASS_ACT_ROOT_JSON_PATH and
    the tables ship inside the NEFF, so hardware evaluates elu.
    Verified on HW: max abs err vs true elu = 7.2e-06.
    """
    from neuronxcc.driver.Job import Job
    from neuronxcc.driver.jobs.support.FindActInfo import findActInfoFile

    src_json = findActInfoFile(Job.getPackageDir(), "gen3")
    src_dir = os.path.dirname(src_json)

    os.makedirs(dst_dir, exist_ok=True)
    for name in os.listdir(src_dir):
        s = os.path.join(src_dir, name)
        if os.path.isfile(s):
            d = os.path.join(dst_dir, name)
            shutil.copy(s, d)
            os.chmod(d, 0o644)

    bkt_path = os.path.join(dst_dir, "exp_and_others_bkt.bin")
    b = np.fromfile(bkt_path, dtype=np.float32).reshape(-1, 8).copy()
    for i in range(781):
        x0 = b[i, 4]
        if i == 777:            # pos_small (|x| < 2^-19): y = x
            b[i, :5] = [0.0, 1.0, 0.0, 0.0, 0.0]
        elif i == 778:          # neg_small: y = x + x^2/2 + x^3/6
            b[i, :5] = [0.0, 1.0, 0.5, 1.0 / 6.0, 0.0]
        elif i == 779:          # pos_large (x > ~88.7): y = x
            b[i, :5] = [0.0, 1.0, 0.0, 0.0, 0.0]
        elif i == 780:          # neg_large: y = -1
            b[i, :5] = [-1.0, 0.0, 0.0, 0.0, 0.0]
        elif x0 < 0:
            b[i, 0] = np.float32(b[i, 0]) - np.float32(1.0)
        elif x0 > 0:
            b[i, :5] = [x0, 1.0, 0.0, 0.0, x0]
    b.tofile(bkt_path)

    prof_path = os.path.join(dst_dir, "exp_and_others.json")
    with open(prof_path) as f:
        prof = json.load(f)
    for ent in prof["profile_meta_data"]:
        if ent["func_name"].startswith("exp"):
            ent["fzero_result"] = 0                    # elu(0) = 0
            ent["fninf_result"] = struct.unpack(
                "<I", struct.pack("<f", -1.0))[0]      # elu(-inf) = -1
    with open(prof_path, "w") as f:
        json.dump(prof, f)

    return os.path.join(dst_dir, "act_info.json")


# ---------------------------------------------------------------------------
def _tile_jobs():
    """Flat device tile order: ('e', tile_idx) / ('n', tile_idx)."""
    jobs = []
    ntile = 0
    for i in range(ETILES):
        jobs.append(("e", i))
        if i % NODE_EVERY == NODE_EVERY - 1:
            jobs.append(("n", ntile))
            ntile += 1
    return jobs


def _emission_order():
    """Per accumulator row (= 512-col block) in device order: the stream
    ('e'/'n') and the block's start col within that stream."""
    rows = []
    for kind, t in _tile_jobs():
        if kind == "e":
            for j in range(3):
                rows.append(("e", t * ZT + j * TZ))
        else:
            w = ZT if t < 8 else TZ
            for j in range(w // TZ):
                rows.append(("n", t * ZT + j * TZ))
    return rows


def _build_program(dt_lo):
    import concourse.tile as tile
    from concourse import bacc
    import concourse.mybir as mybir

    f32 = mybir.dt.float32
    Act = mybir.ActivationFunctionType.Exp  # patched table: evaluates elu

    nc = bacc.Bacc("TRN2", target_bir_lowering=False, debug=False,
                   num_devices=NCORES)

    ea_in = nc.dram_tensor("ea_t", [H, EC], dt_lo, kind="ExternalInput")
    na_in = nc.dram_tensor("na_t", [P, H, NC], dt_lo, kind="ExternalInput")
    we_in = nc.dram_tensor("we_t", [H, H], dt_lo, kind="ExternalInput")
    wp_in = nc.dram_tensor("wp_t", [H, P * H], dt_lo, kind="ExternalInput")
    # one-hot reduce stationaries: oh[s, j] = w_s (x) e_j, s=0 edge, 1 node
    oh_in = nc.dram_tensor("oh_t", [H, 2 * OH * OH], dt_lo,
                           kind="ExternalInput")
    acc_out = nc.dram_tensor("acc_out", [NROWS, TZ], f32,
                             kind="ExternalOutput")

    # edge DMA pieces: 2 z-tiles each (3072 cols, 6KB/partition-row)
    EPIECE = 2 * ZT
    n_epieces = EC // EPIECE  # 41
    # node DMA groups: 3 tiles each (4608 cols x 4 props)
    NPIECE = 3 * ZT
    ngroups = [(0, NPIECE), (NPIECE, NPIECE), (2 * NPIECE, NC - 2 * NPIECE)]

    with tile.TileContext(nc) as tc:
        with (
            tc.tile_pool(name="consts", bufs=1) as cpool,
            tc.tile_pool(name="ework", bufs=6) as epool,
            tc.tile_pool(name="nwork", bufs=2) as npool,
            tc.tile_pool(name="psis", bufs=PIPE + 2) as spool,
            tc.tile_pool(name="outs", bufs=2) as opool,
            tc.tile_pool(name="zpsum", bufs=2, space="PSUM") as zpool,
            tc.tile_pool(name="accpsum", bufs=2, space="PSUM") as apool,
        ):
            # startup-critical DMA order: W_edge, then the first edge
            # columns; everything else rides behind.
            we_sb = cpool.tile([H, H], dt_lo)
            nc.sync.dma_start(we_sb[:], we_in.ap())
            warm = cpool.tile([1, 2], dt_lo)
            nc.scalar.activation(warm[:], we_sb[0:1, 0:2], Act)

            na_parts = {}
            # node sub-DMAs (group gi, prop p, tile tj) hooked onto edge
            # piece loads so they never burst-delay the edge stream
            na_sched = {}
            _slots = {(0, 0): (2, 3), (0, 1): (4, 5), (0, 2): (8, 9),
                      (1, 0): (12, 13), (1, 1): (16, 17), (1, 2): (20, 21),
                      (2, 0): (25, 26), (2, 1): (29, 30), (2, 2): (33, 34)}
            for (gi, tj), (ka, kb) in _slots.items():
                for p in range(P):
                    na_sched.setdefault(ka if p < 2 else kb,
                                        []).append((gi, p, tj))

            def load_nsub(gi, p, tj):
                if gi not in na_parts:
                    nt = npool.tile([H, P, NPIECE], dt_lo, tag="na")
                    na_parts[gi] = nt
                c0, w = ngroups[gi]
                lo = tj * ZT
                hi = min((tj + 1) * ZT, w)
                if lo >= hi:
                    return
                nc.sync.dma_start(na_parts[gi][:, p, lo:hi],
                                  na_in.ap()[p][:, c0 + lo:c0 + hi])

            ea_parts = {}

            def load_epiece(pi, split=False):
                pt = epool.tile([H, EPIECE], dt_lo, tag="ea")
                p0 = pi * EPIECE
                if split:
                    nc.sync.dma_start(pt[:, :TZ], ea_in.ap()[:, p0:p0 + TZ])
                    nc.sync.dma_start(pt[:, TZ:ZT],
                                      ea_in.ap()[:, p0 + TZ:p0 + ZT])
                    nc.sync.dma_start(pt[:, ZT:],
                                      ea_in.ap()[:, p0 + ZT:p0 + EPIECE])
                else:
                    nc.sync.dma_start(pt[:], ea_in.ap()[:, p0:p0 + EPIECE])
                ea_parts[pi] = pt
                for _s in na_sched.pop(pi, ()):
                    load_nsub(*_s)

            load_epiece(0, split=True)

            load_epiece(1)

            oh_sb = cpool.tile([H, 2 * OH, OH], dt_lo)
            nc.sync.dma_start(oh_sb[:], oh_in.ap())

            wp_sb = cpool.tile([H, P, H], dt_lo)
            nc.sync.dma_start(wp_sb[:], wp_in.ap())

            load_epiece(2)
            load_epiece(3)
            load_epiece(4)

            for _k in (2, 3):
                for _s in na_sched.pop(_k, ()):
                    load_nsub(*_s)

            # ---- accumulator row bookkeeping ----
            row = 0          # global 512-block counter
            evac_base = 0    # first row of the current acc bank
            evac_done = 0    # rows already copied+DMA'd out
            acc = apool.tile([H, TZ], f32, tag="acc")

            def evac(upto):
                """Copy acc rows [evac_done, upto) out; split the DMA into
                32-row pieces so it spreads across DMA queues."""
                nonlocal evac_done, evac_base, acc
                lo, hi = evac_done - evac_base, upto - evac_base
                t_sb = opool.tile([H, TZ], f32, tag="tsb")
                nc.vector.tensor_copy(t_sb[lo:hi], acc[lo:hi])
                for r0 in range(lo, hi, OH):
                    r1 = min(r0 + OH, hi)
                    nc.sync.dma_start(
                        acc_out.ap()[evac_base + r0:evac_base + r1],
                        t_sb[r0:r1])
                evac_done = upto
                if upto - evac_base == RPB and upto < NROWS:
                    evac_base = upto
                    acc = apool.tile([H, TZ], f32, tag="acc")

            def emit_reduce(psi, off, kind):
                """One 512-col block: acc[row%RPB] += (w (x) e_j)^T @ psi."""
                nonlocal row
                within = row - evac_base
                base, j = divmod(within, OH)
                oh_idx = (0 if kind == "e" else OH) + j
                nc.tensor.matmul(
                    acc[base * OH:(base + 1) * OH, :],
                    oh_sb[:, oh_idx, :],
                    psi[:, off:off + TZ],
                    start=(j == 0), stop=(j == OH - 1 or row == NROWS - 1),
                    skip_group_check=True,
                )
                row += 1
                within = row - evac_base
                if within == RPB:
                    evac(row)
                elif row == NROWS:
                    # drain the final bank in window pieces so the last
                    # exposed copy+DMA is small
                    evac(row)
                elif row > NROWS - 40 and within % OH == 0:
                    evac(row)

            # ---- software-pipelined tile loop (reduce lags PIPE tiles) ---
            pending = []     # [(psi_tile, width, kind), ...] awaiting reduce

            def flush_one():
                psi, w, kind = pending.pop(0)
                for j in range(w // TZ):
                    emit_reduce(psi, j * TZ, kind)

            for kind, t in _tile_jobs():
                if kind == "e":
                    pi, off = divmod(t * ZT, EPIECE)
                    pt = ea_parts[pi]
                    z = zpool.tile([H, ZT], f32, tag="z")
                    for j in range(3):
                        nc.tensor.matmul(
                            z[:, j * TZ:(j + 1) * TZ], we_sb[:],
                            pt[:, off + j * TZ: off + (j + 1) * TZ],
                            start=True, stop=True)
                    if off + ZT == EPIECE and pi + 5 < n_epieces:
                        load_epiece(pi + 5)   # keep 5 pieces in flight
                    if len(pending) >= PIPE:
                        flush_one()
                    psi = spool.tile([H, ZT], dt_lo, tag="psi")
                    nc.scalar.activation(psi[:], z[:], Act)
                    pending.append((psi, ZT, "e"))
                else:
                    c0 = t * ZT
                    w = ZT if t < 8 else TZ
                    gi, goff = divmod(c0, NPIECE)
                    nt = na_parts[gi]
                    zn = zpool.tile([H, ZT], f32, tag="z")
                    for p in range(P):
                        for j in range(w // TZ):
                            nc.tensor.matmul(
                                zn[:, j * TZ:(j + 1) * TZ], wp_sb[:, p, :],
                                nt[:, p, goff + j * TZ: goff + (j + 1) * TZ],
                                start=(p == 0), stop=(p == P - 1),
                            )
                    if len(pending) >= PIPE:
                        flush_one()
                    psn = spool.tile([H, ZT], dt_lo, tag="psi")
                    nc.scalar.activation(psn[:, :w], zn[:, :w], Act)
                    pending.append((psn, w, "n"))
            while pending:
                flush_one()

    nc.compile()
    return nc


# ---------------------------------------------------------------------------
def kernel(node_attrs, edge_attrs, instruction_batch, distribution,
           node_prop_similarities, relation_similarity,
           W_props, W_edge, w_node_score, w_rel_score,
           edge_indices, node_indices, edge_batch_indices):
    _install_ntff_hook()

    act_root = _build_elu_act_root("/tmp/elu_act_root_v4")
    os.environ["BASS_ACT_ROOT_JSON_PATH"] = act_root

    from concourse import bass_utils
    import concourse.mybir as mybir

    np_lo = np.float16
    dt_lo = mybir.dt.float16

    na = np.asarray(node_attrs, np.float32)
    ea = np.asarray(edge_attrs, np.float32)
    ib = np.asarray(instruction_batch, np.float32)
    dist = np.asarray(distribution, np.float32)
    nps = np.asarray(node_prop_similarities, np.float32)
    rs = np.asarray(relation_similarity, np.float32)
    Wp = np.asarray(W_props, np.float32)
    We = np.asarray(W_edge, np.float32)
    wn = np.asarray(w_node_score, np.float32)
    wr = np.asarray(w_rel_score, np.float32)
    ei = np.asarray(edge_indices).astype(np.int64)
    ni = np.asarray(node_indices).astype(np.int64)
    ebi = np.asarray(edge_batch_indices).astype(np.int64)
    src, dst = ei[0], ei[1]

    # ---- host pre-gating (exact f32, then one fp16 cast) ----
    EPC = E // NCORES  # 125000
    ea_g = (ib[ebi] * ea).astype(np_lo)          # (E, H) fp16
    ea_t = np.zeros((NCORES, H, EC), np_lo)
    ea_t[:, :, :EPC] = np.ascontiguousarray(
        ea_g.reshape(NCORES, EPC, H).transpose(0, 2, 1))
    del ea_g

    NPC = N // NCORES  # 12500
    gate = nps[ni][:, :, None] * ib[ni][:, None, :]   # (N, P, H)
    na_g = (gate * na).astype(np_lo)                  # (N, P, H)
    del gate
    na_t = np.zeros((NCORES, P, H, NC), np_lo)
    na_t[:, :, :, :NPC] = np.ascontiguousarray(
        na_g.reshape(NCORES, NPC, P, H).transpose(0, 2, 3, 1))
    del na_g

    we_t = We.astype(np_lo)                           # (H, H) K=h, M=k
    wp_t = np.ascontiguousarray(
        Wp.transpose(1, 0, 2)).reshape(H, P * H).astype(np_lo)

    # one-hot reduce stationaries: oh[:, s*OH + j, m] = w_s[k] * (m == j)
    oh = np.zeros((H, 2 * OH, OH), np.float32)
    for j in range(OH):
        oh[:, j, j] = wr
        oh[:, OH + j, j] = wn
    oh_t = oh.reshape(H, 2 * OH * OH).astype(np_lo)

    nc = _build_program(dt_lo)

    in_maps = []
    for c in range(NCORES):
        in_maps.append({
            "ea_t": ea_t[c],
            "na_t": na_t[c],
            "we_t": we_t,
            "wp_t": wp_t,
            "oh_t": oh_t,
        })

    res = bass_utils.run_bass_kernel_spmd(
        nc, in_maps, core_ids=list(range(NCORES)),
        trace=bool(os.environ.get("BASS_TRACE")),
        tmpdir=os.environ.get("KERNEL_TRACE_DIR") or None,
    )
    kernel.last_results = res  # for test.py profiling introspection

    # ---- host epilogue ----
    order = _emission_order()
    e_rows = np.array([r for r, (k, _) in enumerate(order) if k == "e"])
    e_cols = np.array([c for k, c in order if k == "e"])
    n_rows = np.array([r for r, (k, _) in enumerate(order) if k == "n"])
    n_cols = np.array([c for k, c in order if k == "n"])

    t_full = np.empty(E, np.float64)
    s_full = np.empty(N, np.float64)
    for c in range(NCORES):
        accv = np.asarray(res.results[c]["acc_out"])  # (NROWS, 512)
        te = np.empty(EC, np.float64)
        te.reshape(-1, TZ)[e_cols // TZ] = accv[e_rows]
        t_full[c * EPC:(c + 1) * EPC] = te[:EPC]
        sn = np.empty(NC, np.float64)
        sn.reshape(-1, TZ)[n_cols // TZ] = accv[n_rows]
        s_full[c * NPC:(c + 1) * NPC] = sn[:NPC]

    # scatter-add edge scalars into nodes, then segment softmaxes
    acc = np.bincount(dst, weights=dist[src].astype(np.float64) * t_full,
                      minlength=N)

    def seg_softmax(x):
        m = np.full(B, -np.inf)
        np.maximum.at(m, ni, x)
        e = np.exp(x - m[ni])
        ssum = np.zeros(B, np.float64)
        np.add.at(ssum, ni, e)
        return e / ssum[ni]

    next_rel = seg_softmax(acc)
    next_states = seg_softmax(s_full)
    rsn = rs[ni].astype(np.float64)
    out = rsn * next_rel + (1.0 - rsn) * next_states
    return out.astype(np.float32)

